# revision 7
# baseline (speedup 1.0000x reference)
"""LIF spike kernel for Trainium2 (Bass/Tile), data-parallel over batch on 8 cores.

Host layout per core: x_core [C=128, T=8, B_loc*HW=4096] f32, one [128, 4096]
tile per timestep. Output spikes uint8 [C, T, 4096], converted to f32 on host.

Exact-f32 math per timestep (THRESH=1, TAU=0.5):
  m_t = r_{t-1} * TAU + x_t     DVE stt  (m_0 = x_0)
  s_t = sign(m_t - 1) -> u8     ACT; -1 saturates to 0 => s_t = (m_t > 1)
  r_t = m_t * [m_t <= 1]        hard reset, split across two engines:
      G_D columns: DVE stt  (m <= 1) * m
      G_P columns: Pool tt  m * mask,  mask = sign(1 - s_t) in {1.0, 0.0} (ACT)

Three compute engines run concurrently: DVE (m-updates + G_D resets),
ACT (spikes + G_P masks, all Sign so no act-table reloads), Pool (G_P resets).
Input DMA on sync queue, output DMA on gpsimd queue.
"""

import numpy as np

import concourse.bacc as bacc
import concourse.mybir as mybir
from concourse.tile import TileContext
from concourse.bass_utils import run_bass_kernel_spmd

B, T, C, H, W = 32, 8, 128, 32, 32
HW = H * W
N_CORES = 8
B_LOC = B // N_CORES
FW = B_LOC * HW  # 4096
TAU = 0.5
THRESH = 1.0

CD = 2048  # G_D columns [0, CD): DVE-reset share
# G_P = [CD, FW) handled by Pool in two slices
P1 = slice(CD, CD + (FW - CD) // 2)
P2 = slice(CD + (FW - CD) // 2, FW)
GD = slice(0, CD)
GDa = slice(0, CD // 2)
GDb = slice(CD // 2, CD)

_nc_cache = None


def build_nc():
    nc = bacc.Bacc("TRN2", target_bir_lowering=False)
    f32 = mybir.dt.float32
    u8 = mybir.dt.uint8
    op = mybir.AluOpType
    AF = mybir.ActivationFunctionType

    x = nc.dram_tensor("x", [C, T, FW], f32, kind="ExternalInput")
    out = nc.dram_tensor("out", [C, T, FW], u8, kind="ExternalOutput")

    with TileContext(nc) as tc:
        with (
            tc.tile_pool(name="xp", bufs=5) as xp,
            tc.tile_pool(name="x0p", bufs=1) as x0p,
            tc.tile_pool(name="mp", bufs=2) as mp,
            tc.tile_pool(name="rp", bufs=2) as rp,
            tc.tile_pool(name="sp", bufs=3) as sp,
            tc.tile_pool(name="kp", bufs=4) as kp,
            tc.tile_pool(name="cp", bufs=1) as cp,
        ):
            bneg1 = cp.tile([C, 1], f32)
            nc.gpsimd.memset(bneg1[:], -1.0)
            bpos1 = cp.tile([C, 1], f32)
            nc.gpsimd.memset(bpos1[:], 1.0)

            def spike(dst_u8, m_ap):
                # u8 spike = saturate(sign(m - 1)) = (m > 1)
                return nc.scalar.activation(
                    dst_u8, m_ap, AF.Sign, bias=bneg1[:], scale=1.0
                )

            def mask_from_s(dst_f32, s_ap):
                # f32 mask = sign(1 - s) in {1.0, 0.0} = (m <= 1), exact
                return nc.scalar.activation(
                    dst_f32, s_ap, AF.Sign, bias=bpos1[:], scale=-1.0
                )

            # --- input DMAs (sync queue), P-slices of x0/x1 first ---
            x0 = x0p.tile([C, FW], f32, tag="x0")
            for sl in (P1, P2, GDa, GDb):
                nc.sync.dma_start(out=x0[:, sl], in_=x[:, 0, sl])
            xts = [x0]
            for t in range(1, T):
                xt = xp.tile([C, FW], f32, tag="x")
                if t == 1:
                    for sl in (P1, P2, GD):
                        nc.sync.dma_start(out=xt[:, sl], in_=x[:, 1, sl])
                else:
                    nc.sync.dma_start(out=xt[:], in_=x[:, t])
                xts.append(xt)

            r_prev = None
            for t in range(T):
                m = xts[0] if t == 0 else mp.tile([C, FW], f32, tag="m")
                st = sp.tile([C, FW], u8, tag="s")

                if t > 0:
                    # DVE m-updates; P-slices first (longest downstream chain)
                    for sl in (P1, P2, GD):
                        nc.vector.scalar_tensor_tensor(
                            m[:, sl], r_prev[:, sl], TAU, xts[t][:, sl],
                            op.mult, op.add,
                        )

                # ACT: spikes + masks (mask right after its s-slice so the
                # Pool reset starts ASAP); s_GD last (no downstream consumer)
                if t < T - 1:
                    mk1 = kp.tile([C, P1.stop - P1.start], f32, tag="k1")
                    mk2 = kp.tile([C, P2.stop - P2.start], f32, tag="k2")
                    spike(st[:, P1], m[:, P1])
                    mask_from_s(mk1[:], st[:, P1])
                    spike(st[:, P2], m[:, P2])
                    mask_from_s(mk2[:], st[:, P2])
                    spike(st[:, GD], m[:, GD])

                    r_new = rp.tile([C, FW], f32, tag="r")
                    # Pool resets for G_P
                    nc.gpsimd.tensor_tensor(r_new[:, P1], m[:, P1], mk1[:], op.mult)
                    nc.gpsimd.tensor_tensor(r_new[:, P2], m[:, P2], mk2[:], op.mult)
                    # DVE reset for G_D
                    nc.vector.scalar_tensor_tensor(
                        r_new[:, GD], m[:, GD], THRESH, m[:, GD], op.is_le, op.mult
                    )
                    r_prev = r_new
                else:
                    # t = T-1: spikes only, finer slices for a short tail
                    for sl in (P1, P2, GDa, GDb):
                        spike(st[:, sl], m[:, sl])

                nc.gpsimd.dma_start(out=out[:, t], in_=st[:])
    nc.compile()
    return nc


def make_in_maps(x: np.ndarray) -> list[dict]:
    xs = np.ascontiguousarray(x).reshape(B, T, C, HW)
    return [
        {
            "x": np.ascontiguousarray(
                xs[i * B_LOC : (i + 1) * B_LOC].transpose(2, 1, 0, 3)
            ).reshape(C, T, FW)
        }
        for i in range(N_CORES)
    ]


def kernel(x: np.ndarray) -> np.ndarray:
    global _nc_cache
    if _nc_cache is None:
        _nc_cache = build_nc()
    res = run_bass_kernel_spmd(_nc_cache, make_in_maps(x), list(range(N_CORES)))
    parts = [
        res.results[i]["out"].reshape(C, T, B_LOC, HW).transpose(2, 1, 0, 3)
        for i in range(N_CORES)
    ]
    full = np.concatenate(parts, axis=0)
    return full.reshape(B, T, C, H, W).astype(np.float32)


# revision 8
# speedup vs baseline: 1.0086x; 1.0086x over previous
"""LIF spike kernel for Trainium2 (Bass/Tile), data-parallel over batch on 8 cores.

Host layout per core: x_core [C=128, T=8, B_loc*HW=4096] f32, one [128, 4096]
tile per timestep. Output spikes uint8 [C, T, 4096], converted to f32 on host.

Exact-f32 math per timestep (THRESH=1, TAU=0.5):
  m_t = r_{t-1} * TAU + x_t     DVE stt  (m_0 = x_0)
  s_t = sign(m_t - 1) -> u8     ACT; -1 saturates to 0 => s_t = (m_t > 1)
  r_t = m_t * [m_t <= 1]        hard reset, split across two engines:
      G_D columns: DVE stt  (m <= 1) * m
      G_P columns: Pool tt  m * mask,  mask = sign(1 - s_t) in {1.0, 0.0} (ACT)

Three compute engines run concurrently: DVE (m-updates + G_D resets),
ACT (spikes + G_P masks, all Sign so no act-table reloads), Pool (G_P resets).
Input DMA on sync queue, output DMA on gpsimd queue.
"""

import numpy as np

import concourse.bacc as bacc
import concourse.mybir as mybir
from concourse.tile import TileContext
from concourse.bass_utils import run_bass_kernel_spmd

B, T, C, H, W = 32, 8, 128, 32, 32
HW = H * W
N_CORES = 8
B_LOC = B // N_CORES
FW = B_LOC * HW  # 4096
TAU = 0.5
THRESH = 1.0

CD = 2048  # G_D columns [0, CD): DVE-reset share
# G_P = [CD, FW) handled by Pool in two slices
P1 = slice(CD, CD + (FW - CD) // 2)
P2 = slice(CD + (FW - CD) // 2, FW)
GD = slice(0, CD)
GDa = slice(0, CD // 2)
GDb = slice(CD // 2, CD)

_nc_cache = None


def build_nc():
    nc = bacc.Bacc("TRN2", target_bir_lowering=False)
    f32 = mybir.dt.float32
    u8 = mybir.dt.uint8
    op = mybir.AluOpType
    AF = mybir.ActivationFunctionType

    x = nc.dram_tensor("x", [C, T, FW], f32, kind="ExternalInput")
    out = nc.dram_tensor("out", [C, T, FW], u8, kind="ExternalOutput")

    with TileContext(nc) as tc:
        with (
            tc.tile_pool(name="xp", bufs=5) as xp,
            tc.tile_pool(name="x0p", bufs=1) as x0p,
            tc.tile_pool(name="mp", bufs=2) as mp,
            tc.tile_pool(name="rp", bufs=2) as rp,
            tc.tile_pool(name="sp", bufs=3) as sp,
            tc.tile_pool(name="kp", bufs=4) as kp,
            tc.tile_pool(name="cp", bufs=1) as cp,
        ):
            bneg1 = cp.tile([C, 1], f32)
            nc.gpsimd.memset(bneg1[:], -1.0)
            bpos1 = cp.tile([C, 1], f32)
            nc.gpsimd.memset(bpos1[:], 1.0)

            def spike(dst_u8, m_ap):
                # u8 spike = saturate(sign(m - 1)) = (m > 1)
                return nc.scalar.activation(
                    dst_u8, m_ap, AF.Sign, bias=bneg1[:], scale=1.0
                )

            def mask_from_s(dst_f32, s_ap):
                # f32 mask = sign(1 - s) in {1.0, 0.0} = (m <= 1), exact
                return nc.scalar.activation(
                    dst_f32, s_ap, AF.Sign, bias=bpos1[:], scale=-1.0
                )

            # --- input DMAs (sync queue), P-slices of x0/x1 first ---
            x0 = x0p.tile([C, FW], f32, tag="x0")
            for sl in (P1, P2, GDa, GDb):
                nc.sync.dma_start(out=x0[:, sl], in_=x[:, 0, sl])
            xts = [x0]
            for t in range(1, T):
                xt = xp.tile([C, FW], f32, tag="x")
                if t == 1:
                    for sl in (P1, P2, GD):
                        nc.sync.dma_start(out=xt[:, sl], in_=x[:, 1, sl])
                else:
                    nc.sync.dma_start(out=xt[:], in_=x[:, t])
                xts.append(xt)

            W1 = P1.stop - P1.start
            W2 = P2.stop - P2.start
            r_prev = None  # (rGD, rP1, rP2) tiles
            for t in range(T):
                m = xts[0] if t == 0 else mp.tile([C, FW], f32, tag="m")
                st = sp.tile([C, FW], u8, tag="s")

                if t > 0:
                    rGD, rP1, rP2 = r_prev
                    # DVE m-updates; P-slices first (longest downstream chain)
                    nc.vector.scalar_tensor_tensor(
                        m[:, P1], rP1[:], TAU, xts[t][:, P1], op.mult, op.add
                    )
                    nc.vector.scalar_tensor_tensor(
                        m[:, P2], rP2[:], TAU, xts[t][:, P2], op.mult, op.add
                    )
                    nc.vector.scalar_tensor_tensor(
                        m[:, GD], rGD[:], TAU, xts[t][:, GD], op.mult, op.add
                    )

                # ACT: spikes + masks (mask right after its s-slice so the
                # Pool reset starts ASAP); s_GD last (no downstream consumer)
                if t < T - 1:
                    mk1 = kp.tile([C, W1], f32, tag="k1")
                    mk2 = kp.tile([C, W2], f32, tag="k2")
                    spike(st[:, P1], m[:, P1])
                    mask_from_s(mk1[:], st[:, P1])
                    spike(st[:, P2], m[:, P2])
                    mask_from_s(mk2[:], st[:, P2])
                    spike(st[:, GD], m[:, GD])

                    # separate r tiles per group: no cross-engine write-write
                    # ordering on a shared tile
                    rGD_n = rp.tile([C, CD], f32, tag="rGD")
                    rP1_n = rp.tile([C, W1], f32, tag="rP1")
                    rP2_n = rp.tile([C, W2], f32, tag="rP2")
                    nc.gpsimd.tensor_tensor(rP1_n[:], m[:, P1], mk1[:], op.mult)
                    nc.gpsimd.tensor_tensor(rP2_n[:], m[:, P2], mk2[:], op.mult)
                    nc.vector.scalar_tensor_tensor(
                        rGD_n[:], m[:, GD], THRESH, m[:, GD], op.is_le, op.mult
                    )
                    r_prev = (rGD_n, rP1_n, rP2_n)
                else:
                    # t = T-1: spikes only, finer slices for a short tail
                    for sl in (P1, P2, GDa, GDb):
                        spike(st[:, sl], m[:, sl])

                nc.gpsimd.dma_start(out=out[:, t], in_=st[:])
    nc.compile()
    return nc


def make_in_maps(x: np.ndarray) -> list[dict]:
    xs = np.ascontiguousarray(x).reshape(B, T, C, HW)
    return [
        {
            "x": np.ascontiguousarray(
                xs[i * B_LOC : (i + 1) * B_LOC].transpose(2, 1, 0, 3)
            ).reshape(C, T, FW)
        }
        for i in range(N_CORES)
    ]


def kernel(x: np.ndarray) -> np.ndarray:
    global _nc_cache
    if _nc_cache is None:
        _nc_cache = build_nc()
    res = run_bass_kernel_spmd(_nc_cache, make_in_maps(x), list(range(N_CORES)))
    parts = [
        res.results[i]["out"].reshape(C, T, B_LOC, HW).transpose(2, 1, 0, 3)
        for i in range(N_CORES)
    ]
    full = np.concatenate(parts, axis=0)
    return full.reshape(B, T, C, H, W).astype(np.float32)


# revision 9
# speedup vs baseline: 1.2030x; 1.1927x over previous
"""LIF spike kernel for Trainium2 (Bass/Tile), data-parallel over batch on 8 cores.

Host layout per core: x_core [C=128, T=8, B_loc*HW=4096] f32, one [128, 4096]
tile per timestep. Output spikes uint8 [C, T, 4096], converted to f32 on host.

Exact-f32 math per timestep (THRESH=1, TAU=0.5):
  m_t = r_{t-1} * TAU + x_t     DVE stt  (m_0 = x_0)
  s_t = sign(m_t - 1) -> u8     ACT engine; -1 saturates to 0 => s_t = (m_t > 1)
  r_t = (m_t <= 1) * m_t        DVE stt (hard reset)

DVE runs the serial 14-stt recurrence (the bottleneck, ~1.04 ns/elem for any
two-tensor-read op); the ACT engine computes all steady-state spikes. At t=0
the input DMA is still cold, so DVE computes the t=0 spikes itself (cheap
one-read tensor_scalar) between graduated r_0 column chunks sized to match
DMA arrival pacing.
"""

import numpy as np

import concourse.bacc as bacc
import concourse.mybir as mybir
from concourse.tile import TileContext
from concourse.bass_utils import run_bass_kernel_spmd

B, T, C, H, W = 32, 8, 128, 32, 32
HW = H * W
N_CORES = 8
B_LOC = B // N_CORES
FW = B_LOC * HW  # 4096
TAU = 0.5
THRESH = 1.0

T0_CHUNKS = (512, 1024, 1024, 1536)  # graduated: small first => early start
T1_SPLIT = 2  # x_1 still streaming in
T7_SPLIT = 4  # short tail

_nc_cache = None


def build_nc():
    nc = bacc.Bacc("TRN2", target_bir_lowering=False)
    f32 = mybir.dt.float32
    u8 = mybir.dt.uint8
    op = mybir.AluOpType
    AF = mybir.ActivationFunctionType

    x = nc.dram_tensor("x", [C, T, FW], f32, kind="ExternalInput")
    out = nc.dram_tensor("out", [C, T, FW], u8, kind="ExternalOutput")

    t0_slices = []
    off = 0
    for w in T0_CHUNKS:
        t0_slices.append(slice(off, off + w))
        off += w
    assert off == FW

    with TileContext(nc) as tc:
        with (
            tc.tile_pool(name="xp", bufs=5) as xp,
            tc.tile_pool(name="x0p", bufs=1) as x0p,
            tc.tile_pool(name="mp", bufs=3) as mp,
            tc.tile_pool(name="rp", bufs=2) as rp,
            tc.tile_pool(name="sp", bufs=3) as sp,
            tc.tile_pool(name="cp", bufs=1) as cp,
        ):
            bneg1 = cp.tile([C, 1], f32)
            nc.gpsimd.memset(bneg1[:], -1.0)

            x0 = x0p.tile([C, FW], f32, tag="x0")
            for sl in t0_slices:
                nc.sync.dma_start(out=x0[:, sl], in_=x[:, 0, sl])
            xts = [x0]
            for t in range(1, T):
                xt = xp.tile([C, FW], f32, tag="x")
                if t == 1:
                    hw1 = FW // T1_SPLIT
                    for j in range(T1_SPLIT):
                        nc.sync.dma_start(
                            out=xt[:, j * hw1 : (j + 1) * hw1],
                            in_=x[:, 1, j * hw1 : (j + 1) * hw1],
                        )
                else:
                    nc.sync.dma_start(out=xt[:], in_=x[:, t])
                xts.append(xt)

            r_prev = None
            for t in range(T):
                if t == 0:
                    m = x0
                    r_prev = rp.tile([C, FW], f32, tag="r")
                    s0 = sp.tile([C, FW], u8, tag="s")
                    # interleave r0/s0 chunks on DVE, paced with DMA arrivals;
                    # act is idle this early (nothing for it to lead on)
                    for sl in t0_slices:
                        nc.vector.scalar_tensor_tensor(
                            r_prev[:, sl], m[:, sl], THRESH, m[:, sl], op.is_le, op.mult
                        )
                        nc.vector.tensor_scalar(
                            s0[:, sl], m[:, sl], THRESH, None, op.is_gt
                        )
                    nc.gpsimd.dma_start(out=out[:, 0], in_=s0[:])
                    continue

                m = mp.tile([C, FW], f32, tag="m")
                st = sp.tile([C, FW], u8, tag="s")
                if t == 1:
                    hw1 = FW // T1_SPLIT
                    r_new = rp.tile([C, FW], f32, tag="r")
                    for j in range(T1_SPLIT):
                        sl = slice(j * hw1, (j + 1) * hw1)
                        nc.vector.scalar_tensor_tensor(
                            m[:, sl], r_prev[:, sl], TAU, xts[t][:, sl], op.mult, op.add
                        )
                        nc.vector.scalar_tensor_tensor(
                            r_new[:, sl], m[:, sl], THRESH, m[:, sl], op.is_le, op.mult
                        )
                        nc.scalar.activation(
                            st[:, sl], m[:, sl], AF.Sign, bias=bneg1[:], scale=1.0
                        )
                        nc.gpsimd.dma_start(out=out[:, t, sl], in_=st[:, sl])
                    r_prev = r_new
                elif t == T - 1:
                    cw7 = FW // T7_SPLIT
                    for j in range(T7_SPLIT):
                        sl = slice(j * cw7, (j + 1) * cw7)
                        nc.vector.scalar_tensor_tensor(
                            m[:, sl], r_prev[:, sl], TAU, xts[t][:, sl], op.mult, op.add
                        )
                        nc.scalar.activation(
                            st[:, sl], m[:, sl], AF.Sign, bias=bneg1[:], scale=1.0
                        )
                        nc.gpsimd.dma_start(out=out[:, t, sl], in_=st[:, sl])
                else:
                    nc.vector.scalar_tensor_tensor(
                        m[:], r_prev[:], TAU, xts[t][:], op.mult, op.add
                    )
                    r_new = rp.tile([C, FW], f32, tag="r")
                    nc.vector.scalar_tensor_tensor(
                        r_new[:], m[:], THRESH, m[:], op.is_le, op.mult
                    )
                    nc.scalar.activation(
                        st[:], m[:], AF.Sign, bias=bneg1[:], scale=1.0
                    )
                    nc.gpsimd.dma_start(out=out[:, t], in_=st[:])
                    r_prev = r_new
    nc.compile()
    return nc


def make_in_maps(x: np.ndarray) -> list[dict]:
    xs = np.ascontiguousarray(x).reshape(B, T, C, HW)
    return [
        {
            "x": np.ascontiguousarray(
                xs[i * B_LOC : (i + 1) * B_LOC].transpose(2, 1, 0, 3)
            ).reshape(C, T, FW)
        }
        for i in range(N_CORES)
    ]


def kernel(x: np.ndarray) -> np.ndarray:
    global _nc_cache
    if _nc_cache is None:
        _nc_cache = build_nc()
    res = run_bass_kernel_spmd(_nc_cache, make_in_maps(x), list(range(N_CORES)))
    parts = [
        res.results[i]["out"].reshape(C, T, B_LOC, HW).transpose(2, 1, 0, 3)
        for i in range(N_CORES)
    ]
    full = np.concatenate(parts, axis=0)
    return full.reshape(B, T, C, H, W).astype(np.float32)


# revision 10
# speedup vs baseline: 1.2087x; 1.0047x over previous
"""LIF spike kernel for Trainium2 (Bass/Tile), data-parallel over batch on 8 cores.

Host layout per core: x_core [C=128, T=8, B_loc*HW=4096] f32, one [128, 4096]
tile per timestep. Output spikes uint8 [C, T, 4096], converted to f32 on host.

Exact-f32 math per timestep (THRESH=1, TAU=0.5):
  m_t = r_{t-1} * TAU + x_t     DVE stt  (m_0 = x_0)
  s_t = sign(m_t - 1) -> u8     ACT engine; -1 saturates to 0 => s_t = (m_t > 1)
  r_t = (m_t <= 1) * m_t        DVE stt (hard reset)

DVE runs the serial 14-stt recurrence (~1.04 ns/elem for any two-tensor-read
op, the bottleneck); ACT computes all spikes. The cold-start is pipelined by
interleaving x0/x1 chunk DMAs and matching the DVE op order to arrival times,
so the chain runs gap-free once the first chunk lands.
"""

import numpy as np

import concourse.bacc as bacc
import concourse.mybir as mybir
from concourse.tile import TileContext
from concourse.bass_utils import run_bass_kernel_spmd

B, T, C, H, W = 32, 8, 128, 32, 32
HW = H * W
N_CORES = 8
B_LOC = B // N_CORES
FW = B_LOC * HW  # 4096
TAU = 0.5
THRESH = 1.0

HALF = FW // 2
T0_CHUNKS = (512, 1536, 1024, 1024)  # c0+c1 == HALF, c2+c3 == HALF
T7_SPLIT = 4

_nc_cache = None


def build_nc():
    nc = bacc.Bacc("TRN2", target_bir_lowering=False)
    f32 = mybir.dt.float32
    u8 = mybir.dt.uint8
    op = mybir.AluOpType
    AF = mybir.ActivationFunctionType

    x = nc.dram_tensor("x", [C, T, FW], f32, kind="ExternalInput")
    out = nc.dram_tensor("out", [C, T, FW], u8, kind="ExternalOutput")

    c0, c1, c2, c3 = T0_CHUNKS
    s_c0 = slice(0, c0)
    s_c1 = slice(c0, c0 + c1)
    s_c2 = slice(HALF, HALF + c2)
    s_c3 = slice(HALF + c2, FW)
    H0 = slice(0, HALF)
    H1 = slice(HALF, FW)

    with TileContext(nc) as tc:
        with (
            tc.tile_pool(name="xp", bufs=5) as xp,
            tc.tile_pool(name="x0p", bufs=1) as x0p,
            tc.tile_pool(name="mp", bufs=3) as mp,
            tc.tile_pool(name="rp", bufs=2) as rp,
            tc.tile_pool(name="sp", bufs=3) as sp,
            tc.tile_pool(name="cp", bufs=1) as cp,
        ):
            bneg1 = cp.tile([C, 1], f32)
            nc.gpsimd.memset(bneg1[:], -1.0)

            # --- input DMAs: interleave x0 chunks with x1 halves so the t=1
            # chain can start before x0 has fully landed ---
            x0 = x0p.tile([C, FW], f32, tag="x0")
            x1 = xp.tile([C, FW], f32, tag="x")
            nc.sync.dma_start(out=x0[:, s_c0], in_=x[:, 0, s_c0])
            nc.sync.dma_start(out=x0[:, s_c1], in_=x[:, 0, s_c1])
            nc.sync.dma_start(out=x1[:, H0], in_=x[:, 1, H0])
            nc.sync.dma_start(out=x0[:, s_c2], in_=x[:, 0, s_c2])
            nc.sync.dma_start(out=x0[:, s_c3], in_=x[:, 0, s_c3])
            nc.sync.dma_start(out=x1[:, H1], in_=x[:, 1, H1])
            xts = [x0, x1]
            for t in range(2, T):
                xt = xp.tile([C, FW], f32, tag="x")
                nc.sync.dma_start(out=xt[:], in_=x[:, t])
                xts.append(xt)

            # --- t=0 and t=1, chunk-pipelined on DVE in arrival order ---
            r0 = rp.tile([C, FW], f32, tag="r")
            s0 = sp.tile([C, FW], u8, tag="s")
            m1 = mp.tile([C, FW], f32, tag="m")
            r1 = rp.tile([C, FW], f32, tag="r")
            s1 = sp.tile([C, FW], u8, tag="s")

            def r_op(dst, src):
                nc.vector.scalar_tensor_tensor(
                    dst, src, THRESH, src, op.is_le, op.mult
                )

            def m_op(dst, r_ap, x_ap):
                nc.vector.scalar_tensor_tensor(dst, r_ap, TAU, x_ap, op.mult, op.add)

            def spike(dst_u8, m_ap):
                nc.scalar.activation(dst_u8, m_ap, AF.Sign, bias=bneg1[:], scale=1.0)

            r_op(r0[:, s_c0], x0[:, s_c0])
            r_op(r0[:, s_c1], x0[:, s_c1])
            spike(s0[:, H0], x0[:, H0])
            m_op(m1[:, H0], r0[:, H0], x1[:, H0])
            r_op(r1[:, H0], m1[:, H0])
            spike(s1[:, H0], m1[:, H0])
            r_op(r0[:, s_c2], x0[:, s_c2])
            r_op(r0[:, s_c3], x0[:, s_c3])
            spike(s0[:, H1], x0[:, H1])
            nc.gpsimd.dma_start(out=out[:, 0], in_=s0[:])
            m_op(m1[:, H1], r0[:, H1], x1[:, H1])
            r_op(r1[:, H1], m1[:, H1])
            spike(s1[:, H1], m1[:, H1])
            nc.gpsimd.dma_start(out=out[:, 1], in_=s1[:])
            r_prev = r1

            # --- steady state t=2..6 ---
            for t in range(2, T - 1):
                m = mp.tile([C, FW], f32, tag="m")
                st = sp.tile([C, FW], u8, tag="s")
                m_op(m[:], r_prev[:], xts[t][:])
                r_new = rp.tile([C, FW], f32, tag="r")
                r_op(r_new[:], m[:])
                spike(st[:], m[:])
                nc.gpsimd.dma_start(out=out[:, t], in_=st[:])
                r_prev = r_new

            # --- t=7 tail, finer slices ---
            m = mp.tile([C, FW], f32, tag="m")
            st = sp.tile([C, FW], u8, tag="s")
            cw7 = FW // T7_SPLIT
            for j in range(T7_SPLIT):
                sl = slice(j * cw7, (j + 1) * cw7)
                m_op(m[:, sl], r_prev[:, sl], xts[T - 1][:, sl])
                spike(st[:, sl], m[:, sl])
                nc.gpsimd.dma_start(out=out[:, T - 1, sl], in_=st[:, sl])
    nc.compile()
    return nc


def make_in_maps(x: np.ndarray) -> list[dict]:
    xs = np.ascontiguousarray(x).reshape(B, T, C, HW)
    return [
        {
            "x": np.ascontiguousarray(
                xs[i * B_LOC : (i + 1) * B_LOC].transpose(2, 1, 0, 3)
            ).reshape(C, T, FW)
        }
        for i in range(N_CORES)
    ]


def kernel(x: np.ndarray) -> np.ndarray:
    global _nc_cache
    if _nc_cache is None:
        _nc_cache = build_nc()
    res = run_bass_kernel_spmd(_nc_cache, make_in_maps(x), list(range(N_CORES)))
    parts = [
        res.results[i]["out"].reshape(C, T, B_LOC, HW).transpose(2, 1, 0, 3)
        for i in range(N_CORES)
    ]
    full = np.concatenate(parts, axis=0)
    return full.reshape(B, T, C, H, W).astype(np.float32)


# revision 12
# speedup vs baseline: 1.2286x; 1.0165x over previous
"""LIF spike kernel for Trainium2 (Bass/Tile), data-parallel over batch on 8 cores.

Host layout per core: x_core [C=128, T=8, B_loc*HW=4096] f32, so each
timestep t is one [128, 4096] tile (16KB contiguous per partition).
Output spikes uint8 [C, T, 4096], converted to f32 on host.

Math per timestep (THRESH=1, TAU=0.5), exact f32 match with reference:
  m_t = r_{t-1} * TAU + x_t          DVE scalar_tensor_tensor   (m_0 = x_0)
  s_t = sign(m_t - 1) -> u8          ACT engine; -1 saturates to 0, so
                                     s_t = (m_t > 1) exactly
  r_t = (m_t <= 1) * m_t             DVE scalar_tensor_tensor (hard reset)

The DVE chain (14 stt ops) is the bottleneck; spikes run on the otherwise
idle ACT engine, output DMA on the tensor engine's queue.
"""

import numpy as np

import concourse.bacc as bacc
import concourse.mybir as mybir
from concourse.tile import TileContext
from concourse.bass_utils import run_bass_kernel_spmd

B, T, C, H, W = 32, 8, 128, 32, 32
HW = H * W
N_CORES = 8
B_LOC = B // N_CORES
FW = B_LOC * HW  # 4096 free width per timestep tile
TAU = 0.5
THRESH = 1.0

T0_SPLIT = 4  # column chunks for t=0 head (starts compute earlier)
T1_SPLIT = 2  # column chunks for t=1 (x_1 still streaming in)
T7_SPLIT = 4  # column chunks for t=7 tail

_nc_cache = None


def build_nc():
    nc = bacc.Bacc("TRN2", target_bir_lowering=False)
    f32 = mybir.dt.float32
    u8 = mybir.dt.uint8
    op = mybir.AluOpType
    AF = mybir.ActivationFunctionType

    x = nc.dram_tensor("x", [C, T, FW], f32, kind="ExternalInput")
    out = nc.dram_tensor("out", [C, T, FW], u8, kind="ExternalOutput")

    with TileContext(nc) as tc:
        with (
            tc.tile_pool(name="xp", bufs=5) as xp,
            tc.tile_pool(name="x0p", bufs=1) as x0p,
            tc.tile_pool(name="mp", bufs=3) as mp,
            tc.tile_pool(name="rp", bufs=2) as rp,
            tc.tile_pool(name="sp", bufs=3) as sp,
            tc.tile_pool(name="cp", bufs=1) as cp,
        ):
            bneg1 = cp.tile([C, 1], f32)
            nc.gpsimd.memset(bneg1[:], -1.0)

            # t=0 arrives as T0_SPLIT column chunks so the chain starts early
            x0 = x0p.tile([C, FW], f32, tag="x0")
            cw = FW // T0_SPLIT
            for j in range(T0_SPLIT):
                nc.sync.dma_start(
                    out=x0[:, j * cw : (j + 1) * cw],
                    in_=x[:, 0, j * cw : (j + 1) * cw],
                )
            xts = [x0]
            for t in range(1, T):
                xt = xp.tile([C, FW], f32, tag="x")
                if t == 1:
                    hw1 = FW // T1_SPLIT
                    for j in range(T1_SPLIT):
                        nc.sync.dma_start(
                            out=xt[:, j * hw1 : (j + 1) * hw1],
                            in_=x[:, 1, j * hw1 : (j + 1) * hw1],
                        )
                else:
                    nc.sync.dma_start(out=xt[:], in_=x[:, t])
                xts.append(xt)

            r_prev = None
            for t in range(T):
                if t == 0:
                    m = x0
                    r_prev = rp.tile([C, FW], f32, tag="r")
                    s0 = sp.tile([C, FW], u8, tag="s")
                    for j in range(T0_SPLIT):
                        sl = slice(j * cw, (j + 1) * cw)
                        nc.vector.scalar_tensor_tensor(
                            r_prev[:, sl], m[:, sl], THRESH, m[:, sl], op.is_le, op.mult
                        )
                        nc.scalar.activation(
                            s0[:, sl], m[:, sl], AF.Sign, bias=bneg1[:], scale=1.0
                        )
                        nc.gpsimd.dma_start(out=out[:, 0, sl], in_=s0[:, sl])
                    continue

                m = mp.tile([C, FW], f32, tag="m")
                st = sp.tile([C, FW], u8, tag="s")
                if t == 1:
                    hw1 = FW // T1_SPLIT
                    r_new = rp.tile([C, FW], f32, tag="r")
                    for j in range(T1_SPLIT):
                        sl = slice(j * hw1, (j + 1) * hw1)
                        nc.vector.scalar_tensor_tensor(
                            m[:, sl], r_prev[:, sl], TAU, xts[t][:, sl], op.mult, op.add
                        )
                        nc.vector.scalar_tensor_tensor(
                            r_new[:, sl], m[:, sl], THRESH, m[:, sl], op.is_le, op.mult
                        )
                        nc.scalar.activation(
                            st[:, sl], m[:, sl], AF.Sign, bias=bneg1[:], scale=1.0
                        )
                        nc.gpsimd.dma_start(out=out[:, t, sl], in_=st[:, sl])
                    r_prev = r_new
                elif t == T - 1:
                    # graduated chunks: small last chunk => short act+DMA tail
                    t7_chunks = (1536, 1024, 1024, 512)
                    off = 0
                    t7_slices = []
                    for w7 in t7_chunks:
                        t7_slices.append(slice(off, off + w7))
                        off += w7
                    for sl in t7_slices:
                        nc.vector.scalar_tensor_tensor(
                            m[:, sl], r_prev[:, sl], TAU, xts[t][:, sl], op.mult, op.add
                        )
                        nc.scalar.activation(
                            st[:, sl], m[:, sl], AF.Sign, bias=bneg1[:], scale=1.0
                        )
                        nc.gpsimd.dma_start(out=out[:, t, sl], in_=st[:, sl])
                else:
                    nc.vector.scalar_tensor_tensor(
                        m[:], r_prev[:], TAU, xts[t][:], op.mult, op.add
                    )
                    r_new = rp.tile([C, FW], f32, tag="r")
                    nc.vector.scalar_tensor_tensor(
                        r_new[:], m[:], THRESH, m[:], op.is_le, op.mult
                    )
                    nc.scalar.activation(
                        st[:], m[:], AF.Sign, bias=bneg1[:], scale=1.0
                    )
                    nc.gpsimd.dma_start(out=out[:, t], in_=st[:])
                    r_prev = r_new
    nc.compile()
    return nc


def make_in_maps(x: np.ndarray) -> list[dict]:
    # x [B, T, C, H, W] -> per core [C, T, B_loc*HW]
    xs = np.ascontiguousarray(x).reshape(B, T, C, HW)
    return [
        {
            "x": np.ascontiguousarray(
                xs[i * B_LOC : (i + 1) * B_LOC].transpose(2, 1, 0, 3)
            ).reshape(C, T, FW)
        }
        for i in range(N_CORES)
    ]


def kernel(x: np.ndarray) -> np.ndarray:
    global _nc_cache
    if _nc_cache is None:
        _nc_cache = build_nc()
    res = run_bass_kernel_spmd(_nc_cache, make_in_maps(x), list(range(N_CORES)))
    # out[c, t, b_loc*HW+hw] -> [b, t, c, hw]
    parts = [
        res.results[i]["out"].reshape(C, T, B_LOC, HW).transpose(2, 1, 0, 3)
        for i in range(N_CORES)
    ]
    full = np.concatenate(parts, axis=0)
    return full.reshape(B, T, C, H, W).astype(np.float32)
